# revision 26
# baseline (speedup 1.0000x reference)
"""Trainium2 Bass kernel for nn_EventTemplateBank (batched 1-D template-bank conv).

Math: score[b,t,e] = sum_{f,l} delayed[e,f,l] * x[b, t+40-l, f] / (L*F),
with delayed = delay-shifted templates (zero fill) and x zero-padded.

Device formulation (per core, data-parallel over batch):
  - Contract over a 128-position window on SBUF partitions.
  - Host pre-permutes x into overlapping-window scratch with one flat
    column axis across the core's 8 batches (683 columns per batch,
    zero-padded to 11*512):
        Xsc[k, f, c] = x[b, 48n + k - 39, f],  c = 683*b + n
    so every output t = 48n + D (D in [0,48)) has its full 80-tap window
    inside the k range of column c.
  - Toeplitz weights (host-built from the tiny templates):
        W[k, s, f, 16d+e] = delayed[e, f, (8s+d) + 79 - k] / 480
    One PSUM tile per D-set s accumulates 6 matmuls (one per feature f):
        out[s][m=(d,e), c-block] += W[:, s, f].T @ Xsc[:, f, c-block]
    Operands are float32r (single-pass PE, ~1 cycle/column at N=512).
  - Output written to DRAM in matmul-native layout; host re-permutes to (B,S,E).
"""

import numpy as np

import concourse.mybir as mybir
from concourse import bacc
from concourse.bass_utils import run_bass_kernel_spmd
from concourse.tile import TileContext, add_dep_helper

# Problem shapes (hardcoded per contract)
B, S, F = 64, 32768, 6
E, L = 16, 80
MAX_DELAY = 10

NCORES = 8
BPC = B // NCORES          # batches per core
Q = 48                     # output positions per rhs column
KWIN = 128                 # contraction window (partitions)
NS = 6                     # D-sets of 8 -> D in [0, 48)
PADF = 39                  # window of column n starts at 48n - 39
NCOLB = (S + Q - 1) // Q   # 683 columns per batch
BLKN = 512                 # columns per matmul block
NBLK = 11                  # ceil(8*683 / 512)
CPAD = NBLK * BLKN         # 5632 padded columns per core
CTOT = BPC * NCOLB         # 5464 real columns per core
LASTN = CTOT - (NBLK - 1) * BLKN   # 344 real columns in the last block

LAST_RESULT = None         # BassKernelResults of the most recent run (for profiling)


def _build_weights(templates: np.ndarray, onset_delays: np.ndarray) -> np.ndarray:
    """W[k, s, f, 16d+e] = delayed[e, f, (8s+d)+79-k] / (L*F), zero outside [0,L)."""
    d = np.round(np.clip(onset_delays, -MAX_DELAY, MAX_DELAY)).astype(np.int64)
    idx = np.arange(L)
    src = idx[None, None, :] - d[:, :, None]                 # (E,F,L)
    valid = (src >= 0) & (src < L)
    delayed = np.take_along_axis(templates, np.clip(src, 0, L - 1), axis=2)
    delayed = np.where(valid, delayed, 0.0).astype(np.float32) / float(L * F)

    D = (8 * np.arange(NS)[:, None] + np.arange(8)[None, :])      # (NS, 8)
    l_idx = D[:, :, None] + 79 - np.arange(KWIN)[None, None, :]   # (NS, 8, K)
    ok = (l_idx >= 0) & (l_idx < L)
    g = delayed[:, :, np.clip(l_idx, 0, L - 1)]                   # (E, F, NS, 8, K)
    g = np.where(ok[None, None], g, 0.0)
    # -> W[k, s, f, dd, e] (k-major so the device DMA is contiguous)
    W = g.transpose(4, 2, 1, 3, 0).reshape(KWIN, NS, F, 128)
    return np.ascontiguousarray(W, dtype=np.float32)


def _build_xsc(x: np.ndarray) -> np.ndarray:
    """Xsc[core, k, f, c] = x[8*core + c//683, 48*(c%683) + k - 39, f], zero OOB/pad."""
    need = Q * (NCOLB - 1) + KWIN
    xpad = np.zeros((B, PADF + need, F), dtype=np.float32)
    xpad[:, PADF:PADF + S, :] = x
    sb, st, sf = xpad.strides
    v = np.lib.stride_tricks.as_strided(
        xpad, shape=(B, KWIN, F, NCOLB), strides=(sb, st, sf, Q * st)
    )
    out = np.zeros((NCORES, KWIN, F, CPAD), dtype=np.float32)
    for b in range(B):
        core, i = divmod(b, BPC)
        out[core, :, :, i * NCOLB:(i + 1) * NCOLB] = v[b]
    return out


def _build_program():
    f32 = mybir.dt.float32
    f32r = mybir.dt.float32r
    nc = bacc.Bacc("TRN2", target_bir_lowering=False, debug=False)
    xsc = nc.dram_tensor("xsc", [KWIN, F, CPAD], f32, kind="ExternalInput")
    w = nc.dram_tensor("w", [KWIN, NS, F, 128], f32, kind="ExternalInput")
    osc = nc.dram_tensor("osc", [NBLK, NS, 128, BLKN], f32, kind="ExternalOutput")

    with TileContext(nc) as tc:
        with (
            tc.tile_pool(name="wp", bufs=1) as wp,
            tc.tile_pool(name="xp", bufs=3) as xp,
            tc.tile_pool(name="pp", bufs=8, space="PSUM") as pp,
            tc.tile_pool(name="op", bufs=6) as op,
        ):
            # First x block queued before the weights so both transfer at t=0
            # and the s=0 weights (smallest piece) gate the first matmul.
            xt0 = xp.tile([KWIN, F * BLKN], f32r, tag="xt")
            prev_dma = nc.gpsimd.dma_start(out=xt0, in_=xsc[:, :, 0:BLKN])
            # Weights: per-set DMA + DVE cast-copy to float32r.
            wt_raw = wp.tile([KWIN, NS * F * 128], f32)
            wt = wp.tile([KWIN, NS * F * 128], f32r)
            wr = w.rearrange("k s f m -> k (s f m)")
            for s in range(NS):
                sl = slice(s * F * 128, (s + 1) * F * 128)
                nc.sync.dma_start(out=wt_raw[:, sl], in_=wr[:, sl])
                nc.vector.tensor_copy(out=wt[:, sl], in_=wt_raw[:, sl])
            for blk in range(NBLK):
                n = BLKN if blk < NBLK - 1 else LASTN
                if blk == 0:
                    xt = xt0
                else:
                    # SWDGE cast-DMA: f32 DRAM -> f32r SBUF (rounds in transfer)
                    xt = xp.tile([KWIN, F * n], f32r, tag="xt")
                    dma = nc.gpsimd.dma_start(
                        out=xt, in_=xsc[:, :, blk * BLKN:blk * BLKN + n]
                    )
                    # Serialize input loads w.r.t. each other: at startup this
                    # gives block 0 the full DMA bandwidth (first matmul gates
                    # on it); in steady state each 4.3us load still hides
                    # under the previous block's ~8us of matmuls.
                    add_dep_helper(dma.ins, prev_dma.ins, reason="serial x loads")
                    prev_dma = dma
                for s in range(NS):
                    ps = pp.tile([128, n], f32, tag="ps")
                    for f in range(F):
                        nc.tensor.matmul(
                            ps,
                            wt[:, (s * F + f) * 128:(s * F + f + 1) * 128],
                            xt[:, f * n:(f + 1) * n],
                            start=(f == 0),
                            stop=(f == F - 1),
                        )
                    ot = op.tile([128, n], f32, tag="ot")
                    nc.vector.tensor_copy(out=ot, in_=ps)
                    nc.sync.dma_start(out=osc[blk, s, :, 0:n], in_=ot)
    nc.compile()   # bacc passes: split multi-waits (HW allows 1 wait/inst), DCE, reg alloc
    return nc


def kernel(x: np.ndarray, templates: np.ndarray, onset_delays: np.ndarray) -> np.ndarray:
    global LAST_RESULT
    x = np.ascontiguousarray(x, dtype=np.float32)
    templates = np.asarray(templates, dtype=np.float32)
    onset_delays = np.asarray(onset_delays, dtype=np.float32)

    W = _build_weights(templates, onset_delays)
    Xsc = _build_xsc(x)                                   # (NCORES, K, F, CPAD)

    nc = _build_program()
    in_maps = [{"xsc": Xsc[c], "w": W} for c in range(NCORES)]
    res = run_bass_kernel_spmd(nc, in_maps, core_ids=list(range(NCORES)))
    LAST_RESULT = res

    osc = np.stack([r["osc"] for r in res.results], axis=0)   # (NCORES,NBLK,NS,128,BLKN)
    o = osc.reshape(NCORES, NBLK, NS, 8, E, BLKN)             # core, blk, s, d, e, n
    o = o.transpose(0, 1, 5, 2, 3, 4)                          # core, blk, n, s, d, e
    o = np.ascontiguousarray(o).reshape(NCORES, CPAD, NS * 8 * E)
    o = o[:, :BPC * NCOLB, :].reshape(NCORES, BPC, NCOLB, NS, 8, E)
    o = o.reshape(B, NCOLB * Q, E)[:, :S, :]
    o = np.ascontiguousarray(o)
    o[:, S - 1, :] = 0.0                                   # reference zero-pads last column
    return o


# revision 31
# speedup vs baseline: 1.1945x; 1.1945x over previous
"""Trainium2 Bass kernel for nn_EventTemplateBank (batched 1-D template-bank conv).

Math: score[b,t,e] = sum_{f,l} delayed[e,f,l] * x[b, t+40-l, f] / (L*F),
with delayed = delay-shifted templates (zero fill) and x zero-padded.

Device formulation (per core, data-parallel over batch):
  - Contract over a 128-position window on SBUF partitions.
  - Host pre-permutes x into overlapping-window scratch with one flat
    column axis across the core's 8 batches (683 columns per batch,
    zero-padded to 11*512):
        Xsc[k, f, c] = x[b, 48n + k - 39, f],  c = 683*b + n
    so every output t = 48n + D (D in [0,48)) has its full 80-tap window
    inside the k range of column c.
  - Toeplitz weights (host-built from the tiny templates):
        W[k, s, f, 16d+e] = delayed[e, f, (8s+d) + 79 - k] / 480
    One PSUM tile per D-set s accumulates 6 matmuls (one per feature f):
        out[s][m=(d,e), c-block] += W[:, s, f].T @ Xsc[:, f, c-block]
    Operands are float32r (single-pass PE, ~1 cycle/column at N=512).
  - Output written to DRAM in matmul-native layout; host re-permutes to (B,S,E).
"""

import numpy as np

import concourse.mybir as mybir
from concourse import bacc
from concourse.bass_utils import run_bass_kernel_spmd
from concourse.tile import TileContext

# Problem shapes (hardcoded per contract)
B, S, F = 64, 32768, 6
E, L = 16, 80
MAX_DELAY = 10

NCORES = 8
BPC = B // NCORES          # batches per core
Q = 48                     # output positions per rhs column
KWIN = 128                 # contraction window (partitions)
NS = 6                     # D-sets of 8 -> D in [0, 48)
PADF = 39                  # window of column n starts at 48n - 39
NCOLB = (S + Q - 1) // Q   # 683 columns per batch
BLKN = 512                 # columns per matmul block
NBLK = 11                  # ceil(8*683 / 512)
CPAD = NBLK * BLKN         # 5632 padded columns per core
CTOT = BPC * NCOLB         # 5464 real columns per core
LASTN = CTOT - (NBLK - 1) * BLKN   # 344 real columns in the last block

LAST_RESULT = None         # BassKernelResults of the most recent run (for profiling)


def _build_weights(templates: np.ndarray, onset_delays: np.ndarray) -> np.ndarray:
    """W[k, s, f, 16d+e] = delayed[e, f, (8s+d)+79-k] / (L*F), zero outside [0,L)."""
    d = np.round(np.clip(onset_delays, -MAX_DELAY, MAX_DELAY)).astype(np.int64)
    idx = np.arange(L)
    src = idx[None, None, :] - d[:, :, None]                 # (E,F,L)
    valid = (src >= 0) & (src < L)
    delayed = np.take_along_axis(templates, np.clip(src, 0, L - 1), axis=2)
    delayed = np.where(valid, delayed, 0.0).astype(np.float32) / float(L * F)

    D = (8 * np.arange(NS)[:, None] + np.arange(8)[None, :])      # (NS, 8)
    l_idx = D[:, :, None] + 79 - np.arange(KWIN)[None, None, :]   # (NS, 8, K)
    ok = (l_idx >= 0) & (l_idx < L)
    g = delayed[:, :, np.clip(l_idx, 0, L - 1)]                   # (E, F, NS, 8, K)
    g = np.where(ok[None, None], g, 0.0)
    # -> W[k, s, f, dd, e] (k-major so the device DMA is contiguous)
    W = g.transpose(4, 2, 1, 3, 0).reshape(KWIN, NS, F, 128)
    return np.ascontiguousarray(W, dtype=np.float32)


def _build_xsc(x: np.ndarray) -> np.ndarray:
    """Xsc[core, k, f, c] = x[8*core + c//683, 48*(c%683) + k - 39, f], zero OOB/pad."""
    need = Q * (NCOLB - 1) + KWIN
    xpad = np.zeros((B, PADF + need, F), dtype=np.float32)
    xpad[:, PADF:PADF + S, :] = x
    sb, st, sf = xpad.strides
    v = np.lib.stride_tricks.as_strided(
        xpad, shape=(B, KWIN, F, NCOLB), strides=(sb, st, sf, Q * st)
    )
    out = np.zeros((NCORES, KWIN, F, CPAD), dtype=np.float32)
    for b in range(B):
        core, i = divmod(b, BPC)
        out[core, :, :, i * NCOLB:(i + 1) * NCOLB] = v[b]
    return out


def _build_program():
    f32 = mybir.dt.float32
    f32r = mybir.dt.float32r
    nc = bacc.Bacc("TRN2", target_bir_lowering=False, debug=False)
    xsc = nc.dram_tensor("xsc", [KWIN, F, CPAD], f32, kind="ExternalInput")
    w = nc.dram_tensor("w", [KWIN, NS, F, 128], f32, kind="ExternalInput")
    osc = nc.dram_tensor("osc", [NBLK, NS, 128, BLKN], f32, kind="ExternalOutput")

    with TileContext(nc) as tc:
        with (
            tc.tile_pool(name="wp", bufs=1) as wp,
            tc.tile_pool(name="xp", bufs=14) as xp,
            tc.tile_pool(name="pp", bufs=8, space="PSUM") as pp,
            tc.tile_pool(name="op", bufs=6) as op,
        ):
            # Weights: per-set DMA + DVE cast-copy to float32r.
            wt_raw = wp.tile([KWIN, NS * F * 128], f32)
            wt = wp.tile([KWIN, NS * F * 128], f32r)
            wr = w.rearrange("k s f m -> k (s f m)")
            for s in range(NS):
                sl = slice(s * F * 128, (s + 1) * F * 128)
                nc.sync.dma_start(out=wt_raw[:, sl], in_=wr[:, sl])
                nc.vector.tensor_copy(out=wt[:, sl], in_=wt_raw[:, sl])
            for blk in range(NBLK):
                n = BLKN if blk < NBLK - 1 else LASTN
                # One SWDGE cast-DMA (f32 DRAM -> f32r SBUF) per feature plane:
                # matmuls gate on single 256KB planes, not the whole 1.5MB block.
                xtp = []
                for f in range(F):
                    xf = xp.tile([KWIN, n], f32r, tag="xtp")
                    nc.gpsimd.dma_start(
                        out=xf, in_=xsc[:, f, blk * BLKN:blk * BLKN + n]
                    )
                    xtp.append(xf)
                pss = [
                    pp.tile([128, n], f32, tag="ps", name=f"ps_{blk}_{s}")
                    for s in range(NS)
                ]
                # f-outer: each arriving plane feeds all 6 accumulating sets.
                for f in range(F):
                    for s in range(NS):
                        nc.tensor.matmul(
                            pss[s],
                            wt[:, (s * F + f) * 128:(s * F + f + 1) * 128],
                            xtp[f],
                            start=(f == 0),
                            stop=(f == F - 1),
                            skip_group_check=True,
                        )
                for s in range(NS):
                    ot = op.tile([128, n], f32, tag="ot")
                    nc.vector.tensor_copy(out=ot, in_=pss[s])
                    nc.sync.dma_start(out=osc[blk, s, :, 0:n], in_=ot)
    nc.compile()   # bacc passes: split multi-waits (HW allows 1 wait/inst), DCE, reg alloc
    return nc


def kernel(x: np.ndarray, templates: np.ndarray, onset_delays: np.ndarray) -> np.ndarray:
    global LAST_RESULT
    x = np.ascontiguousarray(x, dtype=np.float32)
    templates = np.asarray(templates, dtype=np.float32)
    onset_delays = np.asarray(onset_delays, dtype=np.float32)

    W = _build_weights(templates, onset_delays)
    Xsc = _build_xsc(x)                                   # (NCORES, K, F, CPAD)

    nc = _build_program()
    in_maps = [{"xsc": Xsc[c], "w": W} for c in range(NCORES)]
    res = run_bass_kernel_spmd(nc, in_maps, core_ids=list(range(NCORES)))
    LAST_RESULT = res

    osc = np.stack([r["osc"] for r in res.results], axis=0)   # (NCORES,NBLK,NS,128,BLKN)
    o = osc.reshape(NCORES, NBLK, NS, 8, E, BLKN)             # core, blk, s, d, e, n
    o = o.transpose(0, 1, 5, 2, 3, 4)                          # core, blk, n, s, d, e
    o = np.ascontiguousarray(o).reshape(NCORES, CPAD, NS * 8 * E)
    o = o[:, :BPC * NCOLB, :].reshape(NCORES, BPC, NCOLB, NS, 8, E)
    o = o.reshape(B, NCOLB * Q, E)[:, :S, :]
    o = np.ascontiguousarray(o)
    o[:, S - 1, :] = 0.0                                   # reference zero-pads last column
    return o


# revision 32
# speedup vs baseline: 1.2084x; 1.0117x over previous
"""Trainium2 Bass kernel for nn_EventTemplateBank (batched 1-D template-bank conv).

Math: score[b,t,e] = sum_{f,l} delayed[e,f,l] * x[b, t+40-l, f] / (L*F),
with delayed = delay-shifted templates (zero fill) and x zero-padded.

Device formulation (per core, data-parallel over batch):
  - Contract over a 128-position window on SBUF partitions.
  - Host pre-permutes x into overlapping-window scratch with one flat
    column axis across the core's 8 batches (683 columns per batch,
    zero-padded to 11*512):
        Xsc[k, f, c] = x[b, 48n + k - 39, f],  c = 683*b + n
    so every output t = 48n + D (D in [0,48)) has its full 80-tap window
    inside the k range of column c.
  - Toeplitz weights (host-built from the tiny templates):
        W[k, s, f, 16d+e] = delayed[e, f, (8s+d) + 79 - k] / 480
    One PSUM tile per D-set s accumulates 6 matmuls (one per feature f):
        out[s][m=(d,e), c-block] += W[:, s, f].T @ Xsc[:, f, c-block]
    Operands are float32r (single-pass PE, ~1 cycle/column at N=512).
  - Output written to DRAM in matmul-native layout; host re-permutes to (B,S,E).
"""

import numpy as np

import concourse.mybir as mybir
from concourse import bacc
from concourse.bass_utils import run_bass_kernel_spmd
from concourse.tile import TileContext

# Problem shapes (hardcoded per contract)
B, S, F = 64, 32768, 6
E, L = 16, 80
MAX_DELAY = 10

NCORES = 8
BPC = B // NCORES          # batches per core
Q = 48                     # output positions per rhs column
KWIN = 128                 # contraction window (partitions)
NS = 6                     # D-sets of 8 -> D in [0, 48)
PADF = 39                  # window of column n starts at 48n - 39
NCOLB = (S + Q - 1) // Q   # 683 columns per batch
BLKN = 512                 # columns per matmul block
NBLK = 11                  # ceil(8*683 / 512)
CPAD = NBLK * BLKN         # 5632 padded columns per core
CTOT = BPC * NCOLB         # 5464 real columns per core
LASTN = CTOT - (NBLK - 1) * BLKN   # 344 real columns in the last block

LAST_RESULT = None         # BassKernelResults of the most recent run (for profiling)


def _build_weights(templates: np.ndarray, onset_delays: np.ndarray) -> np.ndarray:
    """W[k, s, f, 16d+e] = delayed[e, f, (8s+d)+79-k] / (L*F), zero outside [0,L)."""
    d = np.round(np.clip(onset_delays, -MAX_DELAY, MAX_DELAY)).astype(np.int64)
    idx = np.arange(L)
    src = idx[None, None, :] - d[:, :, None]                 # (E,F,L)
    valid = (src >= 0) & (src < L)
    delayed = np.take_along_axis(templates, np.clip(src, 0, L - 1), axis=2)
    delayed = np.where(valid, delayed, 0.0).astype(np.float32) / float(L * F)

    D = (8 * np.arange(NS)[:, None] + np.arange(8)[None, :])      # (NS, 8)
    l_idx = D[:, :, None] + 79 - np.arange(KWIN)[None, None, :]   # (NS, 8, K)
    ok = (l_idx >= 0) & (l_idx < L)
    g = delayed[:, :, np.clip(l_idx, 0, L - 1)]                   # (E, F, NS, 8, K)
    g = np.where(ok[None, None], g, 0.0)
    # -> W[k, s, f, dd, e] (k-major so the device DMA is contiguous)
    W = g.transpose(4, 2, 1, 3, 0).reshape(KWIN, NS, F, 128)
    return np.ascontiguousarray(W, dtype=np.float32)


def _build_xsc(x: np.ndarray) -> np.ndarray:
    """Xsc[core, k, f, c] = x[8*core + c//683, 48*(c%683) + k - 39, f], zero OOB/pad."""
    need = Q * (NCOLB - 1) + KWIN
    xpad = np.zeros((B, PADF + need, F), dtype=np.float32)
    xpad[:, PADF:PADF + S, :] = x
    sb, st, sf = xpad.strides
    v = np.lib.stride_tricks.as_strided(
        xpad, shape=(B, KWIN, F, NCOLB), strides=(sb, st, sf, Q * st)
    )
    out = np.zeros((NCORES, KWIN, F, CPAD), dtype=np.float32)
    for b in range(B):
        core, i = divmod(b, BPC)
        out[core, :, :, i * NCOLB:(i + 1) * NCOLB] = v[b]
    return out


def _build_program():
    f32 = mybir.dt.float32
    f32r = mybir.dt.float32r
    nc = bacc.Bacc("TRN2", target_bir_lowering=False, debug=False)
    xsc = nc.dram_tensor("xsc", [KWIN, F, CPAD], f32, kind="ExternalInput")
    w = nc.dram_tensor("w", [KWIN, NS, F, 128], f32, kind="ExternalInput")
    osc = nc.dram_tensor("osc", [NBLK, NS, 128, BLKN], f32, kind="ExternalOutput")

    with TileContext(nc) as tc:
        with (
            tc.tile_pool(name="wp", bufs=1) as wp,
            tc.tile_pool(name="xp", bufs=14) as xp,
            tc.tile_pool(name="pp", bufs=8, space="PSUM") as pp,
            tc.tile_pool(name="op", bufs=6) as op,
        ):
            # Weights: per-set DMA + DVE cast-copy to float32r.
            wt_raw = wp.tile([KWIN, NS * F * 128], f32)
            wt = wp.tile([KWIN, NS * F * 128], f32r)
            wr = w.rearrange("k s f m -> k (s f m)")
            for s in range(NS):
                sl = slice(s * F * 128, (s + 1) * F * 128)
                nc.sync.dma_start(out=wt_raw[:, sl], in_=wr[:, sl])
                nc.vector.tensor_copy(out=wt[:, sl], in_=wt_raw[:, sl])
            for blk in range(NBLK):
                n = BLKN if blk < NBLK - 1 else LASTN
                # One SWDGE cast-DMA (f32 DRAM -> f32r SBUF) per feature plane:
                # matmuls gate on single 256KB planes, not the whole 1.5MB block.
                xtp = []
                for f in range(F):
                    xf = xp.tile([KWIN, n], f32r, tag="xtp")
                    nc.gpsimd.dma_start(
                        out=xf, in_=xsc[:, f, blk * BLKN:blk * BLKN + n]
                    )
                    xtp.append(xf)
                pss = [
                    pp.tile([128, n], f32, tag="ps", name=f"ps_{blk}_{s}")
                    for s in range(NS)
                ]

                def evac(s, n=n, blk=blk, pss=pss):
                    ot = op.tile([128, n], f32, tag="ot", name=f"ot_{blk}_{s}")
                    nc.vector.tensor_copy(out=ot, in_=pss[s])
                    nc.sync.dma_start(out=osc[blk, s, :, 0:n], in_=ot)

                if blk == 0:
                    # f-outer: each arriving x-plane feeds all 6 sets, so the
                    # PE starts as soon as the first 256KB plane lands.
                    for f in range(F):
                        for s in range(NS):
                            nc.tensor.matmul(
                                pss[s],
                                wt[:, (s * F + f) * 128:(s * F + f + 1) * 128],
                                xtp[f],
                                start=(f == 0),
                                stop=(f == F - 1),
                                skip_group_check=True,
                            )
                    for s in range(NS):
                        evac(s)
                else:
                    # s-outer: sets complete one after another, so PSUM
                    # evacuation + output DMA stagger across the block.
                    for s in range(NS):
                        for f in range(F):
                            nc.tensor.matmul(
                                pss[s],
                                wt[:, (s * F + f) * 128:(s * F + f + 1) * 128],
                                xtp[f],
                                start=(f == 0),
                                stop=(f == F - 1),
                            )
                        evac(s)
    nc.compile()   # bacc passes: split multi-waits (HW allows 1 wait/inst), DCE, reg alloc
    return nc


def kernel(x: np.ndarray, templates: np.ndarray, onset_delays: np.ndarray) -> np.ndarray:
    global LAST_RESULT
    x = np.ascontiguousarray(x, dtype=np.float32)
    templates = np.asarray(templates, dtype=np.float32)
    onset_delays = np.asarray(onset_delays, dtype=np.float32)

    W = _build_weights(templates, onset_delays)
    Xsc = _build_xsc(x)                                   # (NCORES, K, F, CPAD)

    nc = _build_program()
    in_maps = [{"xsc": Xsc[c], "w": W} for c in range(NCORES)]
    res = run_bass_kernel_spmd(nc, in_maps, core_ids=list(range(NCORES)))
    LAST_RESULT = res

    osc = np.stack([r["osc"] for r in res.results], axis=0)   # (NCORES,NBLK,NS,128,BLKN)
    o = osc.reshape(NCORES, NBLK, NS, 8, E, BLKN)             # core, blk, s, d, e, n
    o = o.transpose(0, 1, 5, 2, 3, 4)                          # core, blk, n, s, d, e
    o = np.ascontiguousarray(o).reshape(NCORES, CPAD, NS * 8 * E)
    o = o[:, :BPC * NCOLB, :].reshape(NCORES, BPC, NCOLB, NS, 8, E)
    o = o.reshape(B, NCOLB * Q, E)[:, :S, :]
    o = np.ascontiguousarray(o)
    o[:, S - 1, :] = 0.0                                   # reference zero-pads last column
    return o
